# revision 32
# baseline (speedup 1.0000x reference)
"""Trainium2 Bass kernel for a single causal attention head (with the
faithful source bug: q = x @ W_key, W_query unused).

Full-input contract: kernel(x, W_key, W_query, W_value) -> [8, 2048, 128].
Sharding: data-parallel over batch B=8 across 8 NeuronCores (1 batch/core).

Per-core math (T=2048, C=1024, H=128):
    K = x @ W_key            (V = x @ W_value)
    S = K @ K.T * H**-0.5    (symmetric since q == k)
    out = softmax(causal(S)) @ V

Schedule highlights:
  - PE clock warmup: dummy wide matmuls fill the input-DMA wait (the
    tensor engine runs ~2x slow for its first ~3us of execution);
    combined with the ACT-table warm moved after the scalar queue's
    DMA triggers, the projections start ~1.5us earlier AND at max
    clock. (Either change alone measures neutral-to-negative; they
    are synergistic.)
  - Projections accumulate K^T/V^T over C in PSUM chasing the input
    DMA; kt casts split scalar/vector inline on the last c-tile.
  - Scores: upper triangle only (S symmetric since q==k); exp without
    max-subtraction in 512-col chunks on ScalarE; causal mask is a
    post-exp multiply on diag tiles; denominators ride the AV matmul
    as a ones-column on V.
  - AV: WIN=3 sliding PSUM window over a shared 8-bank ring, one-round
    software pipeline; rows 14/15 scores+exp hoisted so the endgame
    never waits on ScalarE; last two output tiles normalize/DMA on
    the scalar queue.

Exec ~56-57.5us typical (device clock state varies +-20% between
invocations; exec_time includes ~7.2us fixed preamble and ~2.9us
teardown). Rel err 4e-4. fp8 fails the accuracy gate (simulated).
"""

import numpy as np

import concourse.bass as bass
import concourse.mybir as mybir
import concourse.tile as tile
from concourse import bacc, bass_utils
from concourse.masks import make_identity, make_upper_triangular


P = 128
T = 2048
C = 1024
H = 128
NT = T // P  # 16 seq tiles
NC = C // P  # 8 contraction tiles
NCORES = 8
SCALE = float(H) ** -0.5
F32 = mybir.dt.float32
FP16 = mybir.dt.float16
EXP = mybir.ActivationFunctionType.Exp


def build_module():
    nc = bacc.Bacc(
        "TRN2", target_bir_lowering=False, debug=False, num_devices=NCORES
    )
    xT_d = nc.dram_tensor("xT", [C, T], FP16, kind="ExternalInput").ap()
    w_d = nc.dram_tensor("W", [P, 2, NC, H], FP16, kind="ExternalInput").ap()
    y_d = nc.dram_tensor("y", [T, H], F32, kind="ExternalOutput").ap()

    offs = []
    off = 0
    for j in range(NT):
        offs.append(off)
        off += (NT - j) * P
    e_width = off  # 136 * 128 = 17408

    with tile.TileContext(nc) as tc:
        with (
            tc.tile_pool(name="const", bufs=1) as const,
            tc.tile_pool(name="xt", bufs=8) as xt_pool,
            tc.tile_pool(name="kv", bufs=1) as kv,
            tc.tile_pool(name="e", bufs=1) as e_pool,
            tc.tile_pool(name="outp", bufs=4) as outp,
            tc.tile_pool(name="ps", bufs=8, space="PSUM") as ps,
        ):
            w_sb = const.tile([P, 2, NC, H], FP16)
            nc.sync.dma_start(w_sb[:], w_d[:])
            wk_sb = w_sb[:, 0]
            wv_sb = w_sb[:, 1]

            ident_f = const.tile([P, P], F32)
            make_identity(nc, ident_f)
            dmask_f = const.tile([P, P], F32)
            make_upper_triangular(nc, dmask_f, val=1.0, diag=True)
            ident = const.tile([P, P], FP16)
            nc.vector.tensor_copy(ident[:], ident_f[:])
            dmask = const.tile([P, P], FP16)
            nc.vector.tensor_copy(dmask[:], dmask_f[:])
            ones_f = const.tile([P, 1], F32)
            nc.vector.memset(ones_f[:], 1.0)

            # PE clock warmup: the tensor engine reaches max clock only
            # after ~3us of continuous execution, so the first real
            # matmuls would otherwise run 2x slow. Fill the PE's
            # input-DMA wait with wide dummy matmuls on a scratch tile
            # (result lands in kt_ps[0], which the first real K matmul
            # resets via start=True).
            scratch = const.tile([P, 512], FP16)
            nc.vector.memset(scratch[:], 0.5)

            kt_r = kv.tile([P, T], FP16)
            vt_sb = kv.tile([P, T], FP16)
            vaug = kv.tile([P, NT, P + 1], FP16)
            e_all = e_pool.tile([P, e_width], FP16)

            CHW = 512
            CHN = T // CHW
            kt_ps = [
                ps.tile([P, 512], F32, tag="ps", name=f"ktps{ch}")
                for ch in range(CHN)
            ]
            vt_ps = [
                ps.tile([P, 512], F32, tag="ps", name=f"vtps{ch}")
                for ch in range(CHN)
            ]
            for d in range(8):
                nc.tensor.matmul(
                    kt_ps[0][:],
                    scratch[:, 0:P],
                    scratch[:],
                    start=True,
                    stop=True,
                )
            for c in range(NC):
                xt_c = xt_pool.tile([P, T], FP16, tag="xt", name=f"xt{c}")
                eng = nc.sync if c % 2 else nc.scalar
                eng.dma_start(xt_c[:], xT_d[c * P : (c + 1) * P, :])
                last = c == NC - 1
                for ch in range(CHN):
                    rhs = xt_c[:, ch * CHW : (ch + 1) * CHW]
                    nc.tensor.matmul(
                        kt_ps[ch][:],
                        wk_sb[:, c, :],
                        rhs,
                        start=(c == 0),
                        stop=last,
                    )
                    if last:
                        # kt casts split scalar/vector, emitted as each
                        # chunk's accumulation stops so scores row 0
                        # isn't gated on one serialized cast queue
                        sl = slice(ch * CHW, (ch + 1) * CHW)
                        if ch % 2 == 0:
                            nc.scalar.copy(kt_r[:, sl], kt_ps[ch][:])
                        else:
                            nc.vector.tensor_copy(kt_r[:, sl], kt_ps[ch][:])
                    nc.tensor.matmul(
                        vt_ps[ch][:],
                        wv_sb[:, c, :],
                        rhs,
                        start=(c == 0),
                        stop=last,
                    )
            for ch in range(CHN):
                sl = slice(ch * CHW, (ch + 1) * CHW)
                nc.vector.tensor_copy(vt_sb[:, sl], vt_ps[ch][:])

            # ACT exp-table warm: after the c-loop so the scalar queue's
            # xt DMA triggers run first (xt0's transfer starts ~1.5us
            # earlier); with the PE clock warmup the earlier projection
            # start is at full speed
            warm = const.tile([P, 1], F32)
            nc.vector.memset(warm[:], 0.0)
            nc.scalar.activation(warm[:], warm[:], EXP)

            NAV = P + 1

            def scores_row(j):
                b0 = j * P
                width = T - b0
                pos = 0
                while pos < width:
                    w = min(512, width - pos)
                    s_ps = ps.tile([P, 512], F32, tag="ps", name=f"sps{j}_{pos}")
                    nc.tensor.matmul(
                        s_ps[:, :w],
                        kt_r[:, b0 : b0 + P],
                        kt_r[:, b0 + pos : b0 + pos + w],
                        start=True,
                        stop=True,
                    )
                    nc.scalar.activation(
                        e_all[:, offs[j] + pos : offs[j] + pos + w],
                        s_ps[:, :w],
                        EXP,
                        scale=SCALE,
                    )
                    pos += w
                nc.vector.tensor_mul(
                    e_all[:, offs[j] : offs[j] + P],
                    e_all[:, offs[j] : offs[j] + P],
                    dmask[:],
                )

            nc.vector.memset(vaug[:, :, P : P + 1], 1.0)

            def transpose_v(j):
                vtr = ps.tile([P, 512], FP16, tag="ps", name=f"vtr{j}")
                nc.tensor.transpose(
                    vtr[:, :P], vt_sb[:, j * P : (j + 1) * P], ident[:]
                )
                nc.vector.tensor_copy(vaug[:, j, 0:P], vtr[:, :P])

            WIN = 3
            av_banks = {}

            def av_region(i):
                return av_banks[i][:, :NAV]

            def av_update(j, i, start, stop):
                eji = e_all[
                    :, offs[j] + (i - j) * P : offs[j] + (i - j + 1) * P
                ]
                nc.tensor.matmul(
                    av_region(i), eji, vaug[:, j, :], start=start, stop=stop
                )

            def normalize_out(i):
                av = av_region(i)
                recip = outp.tile([P, 1], F32, tag="recip", name=f"rcp{i}")
                nc.vector.reciprocal(recip[:], av[:, P : P + 1])
                o_sb = outp.tile([P, H], F32, tag="osb", name=f"osb{i}")
                if i >= NT - 2:
                    # ScalarE is idle after the (hoisted) last exps
                    nc.scalar.mul(o_sb[:], av[:, 0:P], recip[:])
                    nc.scalar.dma_start(y_d[i * P : (i + 1) * P, :], o_sb[:])
                else:
                    nc.vector.tensor_scalar_mul(o_sb[:], av[:, 0:P], recip[:])
                    nc.sync.dma_start(y_d[i * P : (i + 1) * P, :], o_sb[:])

            scores_row(0)
            transpose_v(0)
            for j in range(NT):
                # rows 14/15 are hoisted to rounds 11/12 so the endgame
                # AV burst for the last columns never waits on ScalarE
                if j + 1 < NT - 2:
                    scores_row(j + 1)
                    transpose_v(j + 1)
                elif j + 1 < NT:
                    transpose_v(j + 1)
                if j == 11:
                    scores_row(14)
                if j == 12:
                    scores_row(15)
                if j == 0:
                    for i in range(min(WIN, NT)):
                        av_banks[i] = ps.tile(
                            [P, 512], F32, tag="ps", name=f"avb{i}"
                        )
                else:
                    act = j + WIN - 1
                    if act < NT:
                        av_banks[act] = ps.tile(
                            [P, 512], F32, tag="ps", name=f"avb{act}"
                        )
                hi = min(j + WIN, NT) if j == 0 else min(j + WIN - 1, NT)
                for i in range(j, hi):
                    av_update(j, i, start=(j == 0), stop=(j == i))
                if j > 0 and j + WIN - 1 < NT:
                    act = j + WIN - 1
                    for jc in range(j + 1):
                        av_update(jc, act, start=(jc == 0), stop=False)
                normalize_out(j)

    nc.compile()
    return nc


_NC_CACHE = None


def _get_module():
    global _NC_CACHE
    if _NC_CACHE is None:
        _NC_CACHE = build_module()
    return _NC_CACHE


def run(in_maps, trace=False, **kw):
    nc = _get_module()
    return bass_utils.run_bass_kernel_spmd(
        nc, in_maps, core_ids=list(range(NCORES)), trace=trace, **kw
    )


def make_in_maps(x, W_key, W_value):
    x = np.asarray(x, dtype=np.float32).astype(np.float16)
    xT = np.ascontiguousarray(x.transpose(0, 2, 1))
    wk = np.asarray(W_key, np.float32).astype(np.float16)
    wk = wk.reshape(NC, P, H).transpose(1, 0, 2)
    wv = np.asarray(W_value, np.float32).astype(np.float16)
    wv = wv.reshape(NC, P, H).transpose(1, 0, 2)
    w = np.ascontiguousarray(np.stack([wk, wv], axis=1))  # [P, 2, NC, H]
    return [{"xT": xT[b], "W": w} for b in range(NCORES)]


def kernel(x, W_key, W_query, W_value):
    del W_query
    res = run(make_in_maps(x, W_key, W_value), trace=False)
    return np.stack([res.results[b]["y"] for b in range(NCORES)], axis=0)


# revision 33
# speedup vs baseline: 1.0125x; 1.0125x over previous
"""Trainium2 Bass kernel for a single causal attention head (with the
faithful source bug: q = x @ W_key, W_query unused).

Full-input contract: kernel(x, W_key, W_query, W_value) -> [8, 2048, 128].
Sharding: data-parallel over batch B=8 across 8 NeuronCores (1 batch/core).

Per-core math (T=2048, C=1024, H=128):
    K = x @ W_key            (V = x @ W_value)
    S = K @ K.T * H**-0.5    (symmetric since q == k)
    out = softmax(causal(S)) @ V

Schedule highlights:
  - PE clock warmup: dummy wide matmuls fill the input-DMA wait (the
    tensor engine runs ~2x slow for its first ~3us of execution);
    combined with the ACT-table warm moved after the scalar queue's
    DMA triggers, the projections start ~1.5us earlier AND at max
    clock (either change alone is neutral; together ~-1.7us).
  - Projections accumulate K^T/V^T over C in PSUM chasing the input
    DMA; kt casts split scalar/vector inline on the last c-tile.
  - Scores: upper triangle only (S symmetric since q==k); exp without
    max-subtraction in 512-col chunks on ScalarE; causal mask is a
    post-exp multiply on diag tiles; denominators ride the AV matmul
    as a ones-column on V. AV updates interleave between score-chunk
    matmuls so their LDWEIGHTS hide under the 512-col streams.
  - AV: WIN=3 sliding PSUM window over the shared 8-bank ring,
    one-round software pipeline; rows 14/15 scores+exp hoisted so the
    endgame never waits on ScalarE; last two output tiles
    normalize/DMA on the scalar queue.

Exec ~56-57.5us typical (device clock state varies +-20% between
invocations; exec_time includes ~7.2us fixed preamble and ~2.9us
teardown). Rel err 4e-4. fp8 fails the accuracy gate (simulated).
"""

import numpy as np

import concourse.bass as bass
import concourse.mybir as mybir
import concourse.tile as tile
from concourse import bacc, bass_utils
from concourse.masks import make_identity, make_upper_triangular


P = 128
T = 2048
C = 1024
H = 128
NT = T // P  # 16 seq tiles
NC = C // P  # 8 contraction tiles
NCORES = 8
SCALE = float(H) ** -0.5
F32 = mybir.dt.float32
FP16 = mybir.dt.float16
EXP = mybir.ActivationFunctionType.Exp


def build_module():
    nc = bacc.Bacc(
        "TRN2", target_bir_lowering=False, debug=False, num_devices=NCORES
    )
    xT_d = nc.dram_tensor("xT", [C, T], FP16, kind="ExternalInput").ap()
    w_d = nc.dram_tensor("W", [P, 2, NC, H], FP16, kind="ExternalInput").ap()
    y_d = nc.dram_tensor("y", [T, H], F32, kind="ExternalOutput").ap()

    offs = []
    off = 0
    for j in range(NT):
        offs.append(off)
        off += (NT - j) * P
    e_width = off  # 136 * 128 = 17408

    with tile.TileContext(nc) as tc:
        with (
            tc.tile_pool(name="const", bufs=1) as const,
            tc.tile_pool(name="xt", bufs=8) as xt_pool,
            tc.tile_pool(name="kv", bufs=1) as kv,
            tc.tile_pool(name="e", bufs=1) as e_pool,
            tc.tile_pool(name="outp", bufs=4) as outp,
            tc.tile_pool(name="ps", bufs=8, space="PSUM") as ps,
        ):
            w_sb = const.tile([P, 2, NC, H], FP16)
            nc.sync.dma_start(w_sb[:], w_d[:])
            wk_sb = w_sb[:, 0]
            wv_sb = w_sb[:, 1]

            ident_f = const.tile([P, P], F32)
            make_identity(nc, ident_f)
            dmask_f = const.tile([P, P], F32)
            make_upper_triangular(nc, dmask_f, val=1.0, diag=True)
            ident = const.tile([P, P], FP16)
            nc.vector.tensor_copy(ident[:], ident_f[:])
            dmask = const.tile([P, P], FP16)
            nc.vector.tensor_copy(dmask[:], dmask_f[:])
            ones_f = const.tile([P, 1], F32)
            nc.vector.memset(ones_f[:], 1.0)

            # PE clock warmup: the tensor engine reaches max clock only
            # after ~3us of continuous execution, so the first real
            # matmuls would otherwise run 2x slow. Fill the PE's
            # input-DMA wait with wide dummy matmuls on a scratch tile
            # (result lands in kt_ps[0], which the first real K matmul
            # resets via start=True).
            scratch = const.tile([P, 512], FP16)
            nc.vector.memset(scratch[:], 0.5)

            kt_r = kv.tile([P, T], FP16)
            vt_sb = kv.tile([P, T], FP16)
            vaug = kv.tile([P, NT, P + 1], FP16)
            e_all = e_pool.tile([P, e_width], FP16)

            CHW = 512
            CHN = T // CHW
            kt_ps = [
                ps.tile([P, 512], F32, tag="ps", name=f"ktps{ch}")
                for ch in range(CHN)
            ]
            vt_ps = [
                ps.tile([P, 512], F32, tag="ps", name=f"vtps{ch}")
                for ch in range(CHN)
            ]
            for d in range(8):
                nc.tensor.matmul(
                    kt_ps[0][:],
                    scratch[:, 0:P],
                    scratch[:],
                    start=True,
                    stop=True,
                )
            for c in range(NC):
                xt_c = xt_pool.tile([P, T], FP16, tag="xt", name=f"xt{c}")
                eng = nc.sync if c % 2 else nc.scalar
                eng.dma_start(xt_c[:], xT_d[c * P : (c + 1) * P, :])
                last = c == NC - 1
                for ch in range(CHN):
                    rhs = xt_c[:, ch * CHW : (ch + 1) * CHW]
                    nc.tensor.matmul(
                        kt_ps[ch][:],
                        wk_sb[:, c, :],
                        rhs,
                        start=(c == 0),
                        stop=last,
                    )
                    if last:
                        # kt casts split scalar/vector, emitted as each
                        # chunk's accumulation stops so scores row 0
                        # isn't gated on one serialized cast queue
                        sl = slice(ch * CHW, (ch + 1) * CHW)
                        if ch % 2 == 0:
                            nc.scalar.copy(kt_r[:, sl], kt_ps[ch][:])
                        else:
                            nc.vector.tensor_copy(kt_r[:, sl], kt_ps[ch][:])
                    nc.tensor.matmul(
                        vt_ps[ch][:],
                        wv_sb[:, c, :],
                        rhs,
                        start=(c == 0),
                        stop=last,
                    )
            for ch in range(CHN):
                sl = slice(ch * CHW, (ch + 1) * CHW)
                nc.vector.tensor_copy(vt_sb[:, sl], vt_ps[ch][:])

            # ACT exp-table warm: after the c-loop so the scalar queue's
            # xt DMA triggers run first (xt0's transfer starts ~1.5us
            # earlier); with the PE clock warmup the earlier projection
            # start is at full speed
            warm = const.tile([P, 1], F32)
            nc.vector.memset(warm[:], 0.0)
            nc.scalar.activation(warm[:], warm[:], EXP)

            NAV = P + 1

            def scores_row(j, fillers=()):
                # AV updates ride as fillers between score-chunk matmuls
                # so their LDWEIGHTS (~97ns) hide under 512-col streams
                fillers = list(fillers)
                n_fill = len(fillers)
                emitted = 0
                b0 = j * P
                width = T - b0
                n_chunks = -(-width // 512)
                ci = 0
                pos = 0
                while pos < width:
                    w = min(512, width - pos)
                    s_ps = ps.tile([P, 512], F32, tag="ps", name=f"sps{j}_{pos}")
                    nc.tensor.matmul(
                        s_ps[:, :w],
                        kt_r[:, b0 : b0 + P],
                        kt_r[:, b0 + pos : b0 + pos + w],
                        start=True,
                        stop=True,
                    )
                    nc.scalar.activation(
                        e_all[:, offs[j] + pos : offs[j] + pos + w],
                        s_ps[:, :w],
                        EXP,
                        scale=SCALE,
                    )
                    pos += w
                    ci += 1
                    take = (n_fill * ci) // n_chunks - emitted
                    for th in fillers[emitted : emitted + take]:
                        th()
                    emitted += take
                nc.vector.tensor_mul(
                    e_all[:, offs[j] : offs[j] + P],
                    e_all[:, offs[j] : offs[j] + P],
                    dmask[:],
                )

            nc.vector.memset(vaug[:, :, P : P + 1], 1.0)

            def transpose_v(j):
                vtr = ps.tile([P, 512], FP16, tag="ps", name=f"vtr{j}")
                nc.tensor.transpose(
                    vtr[:, :P], vt_sb[:, j * P : (j + 1) * P], ident[:]
                )
                nc.vector.tensor_copy(vaug[:, j, 0:P], vtr[:, :P])

            WIN = 3
            av_banks = {}

            def av_region(i):
                return av_banks[i][:, :NAV]

            def av_update(j, i, start, stop):
                eji = e_all[
                    :, offs[j] + (i - j) * P : offs[j] + (i - j + 1) * P
                ]
                nc.tensor.matmul(
                    av_region(i), eji, vaug[:, j, :], start=start, stop=stop
                )

            def normalize_out(i):
                av = av_region(i)
                recip = outp.tile([P, 1], F32, tag="recip", name=f"rcp{i}")
                nc.vector.reciprocal(recip[:], av[:, P : P + 1])
                o_sb = outp.tile([P, H], F32, tag="osb", name=f"osb{i}")
                if i >= NT - 2:
                    # ScalarE is idle after the (hoisted) last exps
                    nc.scalar.mul(o_sb[:], av[:, 0:P], recip[:])
                    nc.scalar.dma_start(y_d[i * P : (i + 1) * P, :], o_sb[:])
                else:
                    nc.vector.tensor_scalar_mul(o_sb[:], av[:, 0:P], recip[:])
                    nc.sync.dma_start(y_d[i * P : (i + 1) * P, :], o_sb[:])

            scores_row(0)
            transpose_v(0)
            for j in range(NT):
                # build this round's AV updates as thunks (window first,
                # then the newly activated column's catch-up)
                avs = []
                if j == 0:
                    for i in range(min(WIN, NT)):
                        av_banks[i] = ps.tile(
                            [P, 512], F32, tag="ps", name=f"avb{i}"
                        )
                    for i in range(min(WIN, NT)):
                        avs.append(
                            lambda i=i: av_update(
                                0, i, start=True, stop=(i == 0)
                            )
                        )
                else:
                    for i in range(j, min(j + WIN - 1, NT)):
                        avs.append(
                            lambda i=i, j=j: av_update(
                                j, i, start=False, stop=(j == i)
                            )
                        )
                    act = j + WIN - 1
                    if act < NT:
                        av_banks[act] = ps.tile(
                            [P, 512], F32, tag="ps", name=f"avb{act}"
                        )
                        for jc in range(j + 1):
                            avs.append(
                                lambda jc=jc, act=act: av_update(
                                    jc, act, start=(jc == 0), stop=False
                                )
                            )
                # rows 14/15 are hoisted to rounds 11/12 so the endgame
                # AV burst for the last columns never waits on ScalarE
                if j + 1 < NT - 2:
                    scores_row(j + 1, fillers=avs)
                    transpose_v(j + 1)
                    avs = []
                elif j + 1 < NT:
                    transpose_v(j + 1)
                if j == 11:
                    scores_row(14)
                if j == 12:
                    scores_row(15)
                for th in avs:
                    th()
                normalize_out(j)

    nc.compile()
    return nc


_NC_CACHE = None


def _get_module():
    global _NC_CACHE
    if _NC_CACHE is None:
        _NC_CACHE = build_module()
    return _NC_CACHE


def run(in_maps, trace=False, **kw):
    nc = _get_module()
    return bass_utils.run_bass_kernel_spmd(
        nc, in_maps, core_ids=list(range(NCORES)), trace=trace, **kw
    )


def make_in_maps(x, W_key, W_value):
    x = np.asarray(x, dtype=np.float32).astype(np.float16)
    xT = np.ascontiguousarray(x.transpose(0, 2, 1))
    wk = np.asarray(W_key, np.float32).astype(np.float16)
    wk = wk.reshape(NC, P, H).transpose(1, 0, 2)
    wv = np.asarray(W_value, np.float32).astype(np.float16)
    wv = wv.reshape(NC, P, H).transpose(1, 0, 2)
    w = np.ascontiguousarray(np.stack([wk, wv], axis=1))  # [P, 2, NC, H]
    return [{"xT": xT[b], "W": w} for b in range(NCORES)]


def kernel(x, W_key, W_query, W_value):
    del W_query
    res = run(make_in_maps(x, W_key, W_value), trace=False)
    return np.stack([res.results[b]["y"] for b in range(NCORES)], axis=0)
